# revision 21
# baseline (speedup 1.0000x reference)
"""Distributed Trainium2 Bass kernel for nn_NodeFeat (2-hop Chebyshev-style GNN
feature expansion + edge gather), 8 NeuronCores.

Node sharding per the problem's sharding hint:
  - 50000 nodes padded to 50176 = 8 x 6272; core c owns rows [6272c, 6272c+6272).
  - adjacency rows are pre-sorted; each core handles the edges whose ROW is in
    its shard, packed per 128-row dst tile into NCHUNK=18 chunks of 128 slots
    (dummy slots use an out-of-bounds index -> DMA descriptor skipped).
  - pre-phase: each core computes its x3 = [x | x*rsqrt(deg) | x*sqrt(deg)]
    shard in fp16 (per-NODE degree scaling, so no per-edge degree work later);
    one AllGather publishes the fp16 x3_full table.
  - hop1: per dst tile, indirect-DMA gathers fetch the up-to-18x128 neighbor
    rows of x3_full (384B fp16 rows); segment-sum on TensorE via a one-hot
    selector (is_equal of rowloc vs iota) accumulated in PSUM; ScalarE
    evacuates with the 1/deg row scale straight to fp16.
  - one AllGather of the per-core y1 shard (fp16) between hops.
  - hop2: same machinery gathering y1full rows, then minus x3 (kept in SBUF).
  - final: edge endpoints partitioned by owner core (host all-to-all
    bookkeeping); all three tables (x3, y1, xs2) are core-local, so each core
    gathers its endpoint rows locally, transposes [9,64] -> [64,9] on-chip,
    writes packed fp16 rows; the host scatters into the [2,32768,64,9] f32
    output (fp16 -> f32 upcast is exact).

All floating-point math runs on device; the host only shards, pads, reorders
and reassembles (index bookkeeping).
"""
import numpy as np

import concourse.bass as bass
import concourse.mybir as mybir
import concourse.tile as tile
from concourse.bass_utils import run_bass_kernel_spmd

# ---------------- hardcoded problem geometry ----------------
N = 50000
D = 64
EQ = 32768
P = 128
NC = 8                   # cores
NT = 49                  # row tiles per core
NSH = NT * P             # 6272 rows per core
NPAD = NSH * NC          # 50176
NCHUNK = 18              # 128-edge chunks per row tile
FCH = 66                 # final-gather chunks per core (66*128 = 8448 slots)
PC = 6                   # final-gather chunks per piece (11 pieces)
CPAD = 192               # fp16 cols per table row
BIG = 10 ** 7            # out-of-bounds index -> descriptor skipped
F32 = mybir.dt.float32
F16 = mybir.dt.float16
I32 = mybir.dt.int32
EDGE_COLS = NT * NCHUNK  # 882

_prog_cache = {}


class _TC(tile.TileContext):
    """TileContext whose final drain splits sem waits one-per-instruction
    (this walrus rejects >1 sync wait on an instruction)."""

    def _drain_and_barrier(self, tick_clock, wait_clock):
        nc = self.nc
        probe = nc.sync.nop()
        wait_clock.add_sem_waits(
            probe.ins, tile.ScopedClock({None: tick_clock.global_clock}))
        si = probe.ins.sync_info
        waits = list(si.on_wait) if si and si.on_wait else []
        if si is not None:
            si.on_wait = waits[:1]
        for w in waits[1:]:
            n2 = nc.sync.nop()
            if n2.ins.sync_info is None:
                n2.ins.sync_info = mybir.SyncInfo(on_wait=[w], on_update=[])
            else:
                n2.ins.sync_info.on_wait = [w]
        nc.sync.drain()
        nc.all_engine_barrier()
        popped = nc._tile_sem_poison_stack.pop()
        assert popped is self._sem_poison
        nc.clear_and_free_semaphores(list(self.sems.allocated().values()))
        nc.all_engine_barrier()


def _split_multi_waits(nc):
    for fn in nc.m.functions:
        for blk in fn.blocks:
            new_list = []
            for inst in blk.instructions:
                si = inst.sync_info
                waits = list(si.on_wait) if si and si.on_wait else []
                if len(waits) > 1:
                    for j, w in enumerate(waits[:-1]):
                        nop = mybir.InstNoOp(
                            name=f"{inst.name}-ws{j}",
                            engine=inst.engine,
                            ins=[], outs=[],
                            sync_info=mybir.SyncInfo(on_wait=[w], on_update=[]),
                        )
                        nc.register_instruction(nop, overwrite=True)
                        new_list.append(nop)
                    si.on_wait = waits[-1:]
                new_list.append(inst)
            blk.instructions[:] = new_list


def _dims(ap, dims):
    """Same tensor+offset as `ap`, explicit [stride(elem), nelem] dims."""
    return bass.AP(ap.tensor, ap.offset, dims)


def _build_program(ablate=()):
    """ablate: subset of {"pre","hop1","gather1","ag","hop2","gather2",
    "final","gatherf"} to SKIP (perf ablation only — results become wrong)."""
    ab = set(ablate)
    nc = bass.Bass("TRN2", target_bir_lowering=False, debug=False, num_devices=NC)

    x_sh = nc.dram_tensor("x_sh", [NSH, D], F32, kind="ExternalInput")
    degsh_in = nc.dram_tensor("degsh", [P, NT], F32, kind="ExternalInput")
    idx1_in = nc.dram_tensor("idx1", [P, EDGE_COLS], I32, kind="ExternalInput")
    rowloc_in = nc.dram_tensor("rowloc", [P, EDGE_COLS], F16, kind="ExternalInput")
    fidx_loc_in = nc.dram_tensor("fidx_loc", [P, FCH], I32, kind="ExternalInput")
    iota_in = nc.dram_tensor("iota", [P, P], F16, kind="ExternalInput")

    out_f = nc.dram_tensor("out_f", [FCH * P, 576], F16, kind="ExternalOutput")

    x3_bounce = nc.dram_tensor("x3_bounce", [NSH, CPAD], F16)
    x3_full = nc.dram_tensor("x3_full", [NPAD, CPAD], F16, addr_space="Shared")
    y1_bounce = nc.dram_tensor("y1_bounce", [NSH, CPAD], F16)
    y1full = nc.dram_tensor("y1full", [NPAD, CPAD], F16, addr_space="Shared")
    xs2_l = nc.dram_tensor("xs2_l", [NSH, CPAD], F16)

    eq = mybir.AluOpType.is_equal
    mult = mybir.AluOpType.mult
    sub = mybir.AluOpType.subtract
    COPY = mybir.ActivationFunctionType.Copy
    SQRT = mybir.ActivationFunctionType.Sqrt

    with _TC(nc) as tc, nc.allow_low_precision(reason="fp16 tables and matmul operands; PSUM accumulates in f32"), \
            nc.gpsimd.register("bnd_pad") as bnd_pad, \
            nc.gpsimd.register("bnd_sh") as bnd_sh:
        nc.gpsimd.reg_mov(bnd_pad, NPAD - 1)
        nc.gpsimd.reg_mov(bnd_sh, NSH - 1)
        with (
            tc.tile_pool(name="const", bufs=1) as cp,
            tc.tile_pool(name="s", bufs=3) as sp_,
            tc.tile_pool(name="v3", bufs=3) as v3p,
            tc.tile_pool(name="ev", bufs=3) as evp,
            tc.tile_pool(name="x0", bufs=3) as x0p,
            tc.tile_pool(name="v2", bufs=3) as v2p,
            tc.tile_pool(name="g", bufs=2) as gp,
            tc.tile_pool(name="st", bufs=2) as stp,
            tc.tile_pool(name="psum", bufs=4, space="PSUM") as pp,
        ):
            iota_t = cp.tile([P, P], F16)
            nc.sync.dma_start(out=iota_t[:], in_=iota_in[:])
            idx1_t = cp.tile([P, EDGE_COLS], I32)
            nc.sync.dma_start(out=idx1_t[:], in_=idx1_in[:])
            rowloc_t = cp.tile([P, EDGE_COLS], F16)
            nc.sync.dma_start(out=rowloc_t[:], in_=rowloc_in[:])
            degsh_t = cp.tile([P, NT], F32)
            nc.sync.dma_start(out=degsh_t[:], in_=degsh_in[:])
            fidx_loc_t = cp.tile([P, FCH], I32)
            nc.sync.dma_start(out=fidx_loc_t[:], in_=fidx_loc_in[:])

            def build_s(t):
                s_t = sp_.tile([P, NCHUNK, P], F16, tag="s")
                rl = rowloc_t[:, t * NCHUNK:(t + 1) * NCHUNK]
                rl_b = rl.to_broadcast([P, NCHUNK, P])
                io = iota_t[:]
                io_b = _dims(io, [io.ap[0], [0, NCHUNK], io.ap[1]])
                nc.vector.tensor_tensor(out=s_t[:], in0=rl_b, in1=io_b, op=eq)
                return s_t

            def hop_gather(v, table, t):
                for j in range(NCHUNK):
                    col = t * NCHUNK + j
                    nc.gpsimd.indirect_dma_start(
                        out=v[:, j, :], out_offset=None, in_=table[:],
                        in_offset=bass.IndirectOffsetOnAxis(
                            ap=idx1_t[:, col:col + 1], axis=0),
                        bounds_check=bnd_pad, oob_is_err=False,
                    )

            # whole-shard precomputes
            degrev_all = cp.tile([P, NT], F32)
            nc.vector.reciprocal(degrev_all[:], degsh_t[:])
            rq0_all = cp.tile([P, 2, NT], F32)
            nc.scalar.activation(rq0_all[:, 1, :], degsh_t[:], SQRT)
            nc.vector.reciprocal(rq0_all[:, 0, :], rq0_all[:, 1, :])
            # SBUF-resident x3 of the own shard (fp16): [P, NT, 192]
            x3_own = cp.tile([P, NT, 192], F16)

            # ============ pre-phase: x3 = [x | x*rsqrt(deg) | x*sqrt(deg)] ====
            for t in range(NT if "pre" not in ab else 0):
                x_t = x0p.tile([P, D], F32, tag="xt")
                nc.sync.dma_start(out=x_t[:], in_=x_sh[t * P:(t + 1) * P, :])
                nc.scalar.activation(x3_own[:, t, 0:D], x_t[:], COPY)
                b12 = _dims(x3_own[:, t, D:3 * D],
                            [x3_own[:].ap[0], [D, 2], [1, D]])
                xb = _dims(x_t[:], [x_t[:].ap[0], [0, 2], [1, D]])
                rq0b = _dims(rq0_all[:, :, t:t + 1],
                             [rq0_all[:].ap[0], [NT, 2], [0, D]])
                nc.vector.tensor_tensor(out=b12, in0=xb, in1=rq0b, op=mult)
                nc.sync.dma_start(out=x3_bounce[t * P:(t + 1) * P, :],
                                  in_=x3_own[:, t, :])

            # ================= AllGather x3 =================
            if "ag" not in ab:
                nc.gpsimd.collective_compute(
                    "AllGather", mybir.AluOpType.bypass,
                    replica_groups=[list(range(NC))],
                    ins=[x3_bounce[:]], outs=[x3_full[:]],
                )

            # ================= hop 1 =================
            for t in range(NT if "hop1" not in ab else 0):
                v3 = v3p.tile([P, NCHUNK, CPAD], F16, tag="v3")
                if t < 3 or "gather1" in ab:
                    nc.gpsimd.memset(v3[:], 0.0)
                if "gather1" not in ab:
                    hop_gather(v3, x3_full, t)
                s_t = build_s(t)
                ps = pp.tile([P, 192], F32, space="PSUM", tag="ps")
                for j in range(NCHUNK):
                    nc.tensor.matmul(
                        out=ps[:], lhsT=s_t[:, j, :], rhs=v3[:, j, 0:192],
                        start=(j == 0), stop=(j == NCHUNK - 1))
                y1_t = evp.tile([P, 192], F16, tag="y1")
                nc.scalar.activation(y1_t[:], ps[:], COPY,
                                     scale=degrev_all[:, t:t + 1])
                nc.sync.dma_start(out=y1_bounce[t * P:(t + 1) * P, :], in_=y1_t[:])

            # ================= AllGather y1 =================
            if "ag" not in ab:
                nc.gpsimd.collective_compute(
                    "AllGather", mybir.AluOpType.bypass,
                    replica_groups=[list(range(NC))],
                    ins=[y1_bounce[:]], outs=[y1full[:]],
                )

            # ================= hop 2 =================
            for t in range(NT if "hop2" not in ab else 0):
                v2 = v2p.tile([P, NCHUNK, CPAD], F16, tag="v2")
                if t < 3 or "gather2" in ab:
                    nc.gpsimd.memset(v2[:], 0.0)
                if "gather2" not in ab:
                    hop_gather(v2, y1full, t)
                s_t = build_s(t)
                ps = pp.tile([P, 192], F32, space="PSUM", tag="ps")
                for j in range(NCHUNK):
                    nc.tensor.matmul(
                        out=ps[:], lhsT=s_t[:, j, :], rhs=v2[:, j, 0:192],
                        start=(j == 0), stop=(j == NCHUNK - 1))
                tmp = evp.tile([P, 192], F16, tag="tmp2")
                nc.scalar.activation(tmp[:], ps[:], COPY,
                                     scale=degrev_all[:, t:t + 1])
                xs2_t = evp.tile([P, 192], F16, tag="xs2")
                nc.vector.tensor_tensor(out=xs2_t[:], in0=tmp[:],
                                        in1=x3_own[:, t, :], op=sub)
                nc.sync.dma_start(out=xs2_l[t * P:(t + 1) * P, :], in_=xs2_t[:])

            # ================= final gather + transpose =================
            tables = [x3_bounce, y1_bounce, xs2_l]
            for pc_i in range(FCH // PC if "final" not in ab else 0):
                gs = []
                for h in range(3):
                    g = gp.tile([P, PC, CPAD], F16, tag=f"g{h}")
                    if pc_i < 2 or "gatherf" in ab:
                        nc.gpsimd.memset(g[:], 0.0)
                    if "gatherf" not in ab:
                        for j in range(PC):
                            col = pc_i * PC + j
                            nc.gpsimd.indirect_dma_start(
                                out=g[:, j, :], out_offset=None,
                                in_=tables[h][:],
                                in_offset=bass.IndirectOffsetOnAxis(
                                    ap=fidx_loc_t[:, col:col + 1], axis=0),
                                bounds_check=bnd_sh, oob_is_err=False,
                            )
                    gs.append(g)
                stage = stp.tile([P, PC, 576], F16, tag="stage")
                for k in range(9):
                    h, b = divmod(k, 3)
                    src = gs[h][:, :, b * D:(b + 1) * D]
                    dst = _dims(stage[:, :, k:k + 1],
                                [stage[:].ap[0], [576, PC], [9, D]])
                    if k % 2 == 0:
                        nc.vector.tensor_copy(out=dst, in_=src)
                    else:
                        nc.scalar.activation(dst, src, COPY)
                obase = out_f[pc_i * PC * P:(pc_i + 1) * PC * P, :]
                orows = _dims(obase, [[576, P], [P * 576, PC], [1, 576]])
                nc.sync.dma_start(out=orows, in_=stage[:])

    _split_multi_waits(nc)
    return nc


def _plan(x, deg, adj_row, adj_col, edge):
    """Host-side sharding: pure index bookkeeping + input reordering."""
    x = np.asarray(x, np.float32)
    deg = np.asarray(deg, np.float32).reshape(-1)
    adj_row = np.asarray(adj_row, np.int64)
    adj_col = np.asarray(adj_col, np.int64)
    edge = np.asarray(edge, np.int64)

    iota_np = np.tile(np.arange(P, dtype=np.float16), (P, 1))
    ep = edge.reshape(-1)

    in_maps, positions = [], []
    for c in range(NC):
        r0 = c * NSH
        idx1 = np.full((P, EDGE_COLS), BIG, np.int32)
        rowloc = np.full((P, EDGE_COLS), -1.0, np.float16)
        for t in range(NT):
            base = r0 + t * P
            lo = np.searchsorted(adj_row, base, side="left")
            hi = np.searchsorted(adj_row, base + P, side="left")
            n_e = hi - lo
            assert n_e <= NCHUNK * P, f"tile overflow: {n_e}"
            sl = np.arange(n_e)
            jj, pp_ = divmod(sl, P)
            colbase = t * NCHUNK
            idx1[pp_, colbase + jj] = adj_col[lo:hi]
            rowloc[pp_, colbase + jj] = (adj_row[lo:hi] - base).astype(np.float16)
        real = min(NSH, max(0, N - r0))
        dlocal = np.ones(NSH, np.float32)
        dlocal[:real] = deg[r0:r0 + real]
        degsh = dlocal.reshape(NT, P).T.copy()

        x_shard = np.zeros((NSH, D), np.float32)
        x_shard[:real] = x[r0:r0 + real]

        mine = np.nonzero((ep >= r0) & (ep < r0 + NSH))[0]
        n_c = len(mine)
        assert n_c <= FCH * P, f"endpoint overflow: {n_c}"
        fidx_loc = np.full((P, FCH), BIG, np.int32)
        sl = np.arange(n_c)
        jj, pp_ = divmod(sl, P)
        fidx_loc[pp_, jj] = (ep[mine] - r0).astype(np.int32)
        positions.append(mine)

        in_maps.append({
            "x_sh": x_shard,
            "degsh": degsh,
            "idx1": idx1,
            "rowloc": rowloc,
            "fidx_loc": fidx_loc,
            "iota": iota_np,
        })
    return in_maps, positions


def _assemble(results, positions):
    out = np.zeros((2 * EQ, 576), np.float32)
    for c in range(NC):
        rows = results[c]["out_f"]
        n_c = len(positions[c])
        out[positions[c]] = rows[:n_c].astype(np.float32)
    return out.reshape(2, EQ, D, 9)


def kernel(x, deg, adj_row, adj_col, edge):
    import time
    if "nc" not in _prog_cache:
        t0 = time.time()
        _prog_cache["nc"] = _build_program()
        print(f"[kernel] program build: {time.time()-t0:.1f}s", flush=True)
    nc = _prog_cache["nc"]
    t0 = time.time()
    in_maps, positions = _plan(x, deg, adj_row, adj_col, edge)
    print(f"[kernel] host plan: {time.time()-t0:.1f}s", flush=True)
    t0 = time.time()
    res = run_bass_kernel_spmd(nc, in_maps, list(range(NC)))
    print(f"[kernel] compile+run: {time.time()-t0:.1f}s", flush=True)
    return _assemble(res.results, positions)


# revision 22
# speedup vs baseline: 1.0379x; 1.0379x over previous
"""Distributed Trainium2 Bass kernel for nn_NodeFeat (2-hop Chebyshev-style GNN
feature expansion + edge gather), 8 NeuronCores.

Node sharding per the problem's sharding hint:
  - 50000 nodes padded to 50176 = 8 x 6272; core c owns rows [6272c, 6272c+6272).
  - adjacency rows are pre-sorted; each core handles the edges whose ROW is in
    its shard, packed per 128-row dst tile into NCHUNK=18 chunks of 128 slots
    (dummy slots use an out-of-bounds index -> DMA descriptor skipped).
  - pre-phase: each core computes its x3 = [x | x*rsqrt(deg) | x*sqrt(deg)]
    shard in fp16 (per-NODE degree scaling, so no per-edge degree work later);
    one AllGather publishes the fp16 x3_full table.
  - hop1: per dst tile, indirect-DMA gathers fetch the up-to-18x128 neighbor
    rows of x3_full (384B fp16 rows); segment-sum on TensorE via a one-hot
    selector (is_equal of rowloc vs iota) accumulated in PSUM; ScalarE
    evacuates with the 1/deg row scale straight to fp16.
  - one AllGather of the per-core y1 shard (fp16) between hops.
  - hop2: same machinery gathering y1full rows, then minus x3 (kept in SBUF).
  - final: edge endpoints partitioned by owner core (host all-to-all
    bookkeeping); all three tables (x3, y1, xs2) are core-local, so each core
    gathers its endpoint rows locally, transposes [9,64] -> [64,9] on-chip,
    writes packed fp16 rows; the host scatters into the [2,32768,64,9] f32
    output (fp16 -> f32 upcast is exact).

All floating-point math runs on device; the host only shards, pads, reorders
and reassembles (index bookkeeping).
"""
import numpy as np

import concourse.bass as bass
import concourse.mybir as mybir
import concourse.tile as tile
from concourse.bass_utils import run_bass_kernel_spmd

# ---------------- hardcoded problem geometry ----------------
N = 50000
D = 64
EQ = 32768
P = 128
NC = 8                   # cores
NT = 49                  # row tiles per core
NSH = NT * P             # 6272 rows per core
NPAD = NSH * NC          # 50176
NCHUNK = 18              # 128-edge chunks per row tile
FCH = 66                 # final-gather chunks per core (66*128 = 8448 slots)
PC = 6                   # final-gather chunks per piece (11 pieces)
CPAD = 192               # fp16 cols per table row
BIG = 10 ** 7            # out-of-bounds index -> descriptor skipped
F32 = mybir.dt.float32
F16 = mybir.dt.float16
I32 = mybir.dt.int32
EDGE_COLS = NT * NCHUNK  # 882

_prog_cache = {}


class _TC(tile.TileContext):
    """TileContext whose final drain splits sem waits one-per-instruction
    (this walrus rejects >1 sync wait on an instruction)."""

    def _drain_and_barrier(self, tick_clock, wait_clock):
        nc = self.nc
        probe = nc.sync.nop()
        wait_clock.add_sem_waits(
            probe.ins, tile.ScopedClock({None: tick_clock.global_clock}))
        si = probe.ins.sync_info
        waits = list(si.on_wait) if si and si.on_wait else []
        if si is not None:
            si.on_wait = waits[:1]
        for w in waits[1:]:
            n2 = nc.sync.nop()
            if n2.ins.sync_info is None:
                n2.ins.sync_info = mybir.SyncInfo(on_wait=[w], on_update=[])
            else:
                n2.ins.sync_info.on_wait = [w]
        nc.sync.drain()
        nc.all_engine_barrier()
        popped = nc._tile_sem_poison_stack.pop()
        assert popped is self._sem_poison
        nc.clear_and_free_semaphores(list(self.sems.allocated().values()))
        nc.all_engine_barrier()


def _split_multi_waits(nc):
    for fn in nc.m.functions:
        for blk in fn.blocks:
            new_list = []
            for inst in blk.instructions:
                si = inst.sync_info
                waits = list(si.on_wait) if si and si.on_wait else []
                if len(waits) > 1:
                    for j, w in enumerate(waits[:-1]):
                        nop = mybir.InstNoOp(
                            name=f"{inst.name}-ws{j}",
                            engine=inst.engine,
                            ins=[], outs=[],
                            sync_info=mybir.SyncInfo(on_wait=[w], on_update=[]),
                        )
                        nc.register_instruction(nop, overwrite=True)
                        new_list.append(nop)
                    si.on_wait = waits[-1:]
                new_list.append(inst)
            blk.instructions[:] = new_list


def _dims(ap, dims):
    """Same tensor+offset as `ap`, explicit [stride(elem), nelem] dims."""
    return bass.AP(ap.tensor, ap.offset, dims)


def _build_program(ablate=()):
    """ablate: subset of {"pre","hop1","gather1","ag","hop2","gather2",
    "final","gatherf"} to SKIP (perf ablation only — results become wrong)."""
    ab = set(ablate)
    nc = bass.Bass("TRN2", target_bir_lowering=False, debug=False, num_devices=NC)

    x_sh = nc.dram_tensor("x_sh", [NSH, D], F32, kind="ExternalInput")
    degsh_in = nc.dram_tensor("degsh", [P, NT], F32, kind="ExternalInput")
    idx1_in = nc.dram_tensor("idx1", [P, EDGE_COLS], I32, kind="ExternalInput")
    rowloc_in = nc.dram_tensor("rowloc", [P, EDGE_COLS], F16, kind="ExternalInput")
    fidx_loc_in = nc.dram_tensor("fidx_loc", [P, FCH], I32, kind="ExternalInput")
    iota_in = nc.dram_tensor("iota", [P, P], F16, kind="ExternalInput")

    out_f = nc.dram_tensor("out_f", [FCH * P, 576], F16, kind="ExternalOutput")

    x3_bounce = nc.dram_tensor("x3_bounce", [NSH, CPAD], F16)
    x3_full = nc.dram_tensor("x3_full", [NPAD, CPAD], F16, addr_space="Shared")
    y1_bounce = nc.dram_tensor("y1_bounce", [NSH, CPAD], F16)
    y1full = nc.dram_tensor("y1full", [NPAD, CPAD], F16, addr_space="Shared")
    h_l = nc.dram_tensor("h_l", [NSH, 576], F16)

    eq = mybir.AluOpType.is_equal
    mult = mybir.AluOpType.mult
    sub = mybir.AluOpType.subtract
    COPY = mybir.ActivationFunctionType.Copy
    SQRT = mybir.ActivationFunctionType.Sqrt

    with _TC(nc) as tc, nc.allow_low_precision(reason="fp16 tables and matmul operands; PSUM accumulates in f32"), \
            nc.gpsimd.register("bnd_pad") as bnd_pad, \
            nc.gpsimd.register("bnd_sh") as bnd_sh:
        nc.gpsimd.reg_mov(bnd_pad, NPAD - 1)
        nc.gpsimd.reg_mov(bnd_sh, NSH - 1)
        with (
            tc.tile_pool(name="const", bufs=1) as cp,
            tc.tile_pool(name="s", bufs=4) as sp_,
            tc.tile_pool(name="v3", bufs=4) as v3p,
            tc.tile_pool(name="ev", bufs=3) as evp,
            tc.tile_pool(name="x0", bufs=3) as x0p,
            tc.tile_pool(name="v2", bufs=4) as v2p,
            tc.tile_pool(name="g", bufs=3) as gp,
            tc.tile_pool(name="st", bufs=3) as stp,
            tc.tile_pool(name="psum", bufs=4, space="PSUM") as pp,
        ):
            iota_t = cp.tile([P, P], F16)
            nc.sync.dma_start(out=iota_t[:], in_=iota_in[:])
            idx1_t = cp.tile([P, EDGE_COLS], I32)
            nc.sync.dma_start(out=idx1_t[:], in_=idx1_in[:])
            rowloc_t = cp.tile([P, EDGE_COLS], F16)
            nc.sync.dma_start(out=rowloc_t[:], in_=rowloc_in[:])
            degsh_t = cp.tile([P, NT], F32)
            nc.sync.dma_start(out=degsh_t[:], in_=degsh_in[:])
            fidx_loc_t = cp.tile([P, FCH], I32)
            nc.sync.dma_start(out=fidx_loc_t[:], in_=fidx_loc_in[:])

            def build_s(t):
                s_t = sp_.tile([P, NCHUNK, P], F16, tag="s")
                rl = rowloc_t[:, t * NCHUNK:(t + 1) * NCHUNK]
                rl_b = rl.to_broadcast([P, NCHUNK, P])
                io = iota_t[:]
                io_b = _dims(io, [io.ap[0], [0, NCHUNK], io.ap[1]])
                nc.vector.tensor_tensor(out=s_t[:], in0=rl_b, in1=io_b, op=eq)
                return s_t

            def hop_gather(v, table, t):
                for j in range(NCHUNK):
                    col = t * NCHUNK + j
                    nc.gpsimd.indirect_dma_start(
                        out=v[:, j, :], out_offset=None, in_=table[:],
                        in_offset=bass.IndirectOffsetOnAxis(
                            ap=idx1_t[:, col:col + 1], axis=0),
                        bounds_check=bnd_pad, oob_is_err=False,
                    )

            # whole-shard precomputes
            degrev_all = cp.tile([P, NT], F32)
            nc.vector.reciprocal(degrev_all[:], degsh_t[:])
            rq0_all = cp.tile([P, 2, NT], F32)
            nc.scalar.activation(rq0_all[:, 1, :], degsh_t[:], SQRT)
            nc.vector.reciprocal(rq0_all[:, 0, :], rq0_all[:, 1, :])
            # SBUF-resident x3 of the own shard (fp16): [P, NT, 192]
            x3_own = cp.tile([P, NT, 192], F16)

            # ============ pre-phase: x3 = [x | x*rsqrt(deg) | x*sqrt(deg)] ====
            for t in range(NT if "pre" not in ab else 0):
                x_t = x0p.tile([P, D], F32, tag="xt")
                nc.sync.dma_start(out=x_t[:], in_=x_sh[t * P:(t + 1) * P, :])
                nc.scalar.activation(x3_own[:, t, 0:D], x_t[:], COPY)
                b12 = _dims(x3_own[:, t, D:3 * D],
                            [x3_own[:].ap[0], [D, 2], [1, D]])
                xb = _dims(x_t[:], [x_t[:].ap[0], [0, 2], [1, D]])
                rq0b = _dims(rq0_all[:, :, t:t + 1],
                             [rq0_all[:].ap[0], [NT, 2], [0, D]])
                nc.vector.tensor_tensor(out=b12, in0=xb, in1=rq0b, op=mult)
                nc.sync.dma_start(out=x3_bounce[t * P:(t + 1) * P, :],
                                  in_=x3_own[:, t, :])
                h0 = _dims(h_l[t * P:(t + 1) * P, 0:CPAD],
                           [[576, P], [1, CPAD]])
                nc.sync.dma_start(out=h0, in_=x3_own[:, t, :])

            # ================= AllGather x3 =================
            if "ag" not in ab:
                nc.gpsimd.collective_compute(
                    "AllGather", mybir.AluOpType.bypass,
                    replica_groups=[list(range(NC))],
                    ins=[x3_bounce[:]], outs=[x3_full[:]],
                )

            # ================= hop 1 =================
            for t in range(NT if "hop1" not in ab else 0):
                v3 = v3p.tile([P, NCHUNK, CPAD], F16, tag="v3")
                if t < 4 or "gather1" in ab:
                    nc.gpsimd.memset(v3[:], 0.0)
                if "gather1" not in ab:
                    hop_gather(v3, x3_full, t)
                s_t = build_s(t)
                ps = pp.tile([P, 192], F32, space="PSUM", tag="ps")
                for j in range(NCHUNK):
                    nc.tensor.matmul(
                        out=ps[:], lhsT=s_t[:, j, :], rhs=v3[:, j, 0:192],
                        start=(j == 0), stop=(j == NCHUNK - 1))
                y1_t = evp.tile([P, 192], F16, tag="y1")
                nc.scalar.activation(y1_t[:], ps[:], COPY,
                                     scale=degrev_all[:, t:t + 1])
                nc.sync.dma_start(out=y1_bounce[t * P:(t + 1) * P, :], in_=y1_t[:])
                h1 = _dims(h_l[t * P:(t + 1) * P, CPAD:2 * CPAD],
                           [[576, P], [1, CPAD]])
                nc.sync.dma_start(out=h1, in_=y1_t[:])

            # ================= AllGather y1 =================
            if "ag" not in ab:
                nc.gpsimd.collective_compute(
                    "AllGather", mybir.AluOpType.bypass,
                    replica_groups=[list(range(NC))],
                    ins=[y1_bounce[:]], outs=[y1full[:]],
                )

            # ================= hop 2 =================
            for t in range(NT if "hop2" not in ab else 0):
                v2 = v2p.tile([P, NCHUNK, CPAD], F16, tag="v2")
                if t < 4 or "gather2" in ab:
                    nc.gpsimd.memset(v2[:], 0.0)
                if "gather2" not in ab:
                    hop_gather(v2, y1full, t)
                s_t = build_s(t)
                ps = pp.tile([P, 192], F32, space="PSUM", tag="ps")
                for j in range(NCHUNK):
                    nc.tensor.matmul(
                        out=ps[:], lhsT=s_t[:, j, :], rhs=v2[:, j, 0:192],
                        start=(j == 0), stop=(j == NCHUNK - 1))
                tmp = evp.tile([P, 192], F16, tag="tmp2")
                nc.scalar.activation(tmp[:], ps[:], COPY,
                                     scale=degrev_all[:, t:t + 1])
                xs2_t = evp.tile([P, 192], F16, tag="xs2")
                nc.vector.tensor_tensor(out=xs2_t[:], in0=tmp[:],
                                        in1=x3_own[:, t, :], op=sub)
                h2 = _dims(h_l[t * P:(t + 1) * P, 2 * CPAD:3 * CPAD],
                           [[576, P], [1, CPAD]])
                nc.sync.dma_start(out=h2, in_=xs2_t[:])

            # ================= final gather + transpose =================
            for pc_i in range(FCH // PC if "final" not in ab else 0):
                g = gp.tile([P, PC, 576], F16, tag="g")
                if pc_i < 3 or "gatherf" in ab:
                    nc.gpsimd.memset(g[:], 0.0)
                if "gatherf" not in ab:
                    for j in range(PC):
                        col = pc_i * PC + j
                        nc.gpsimd.indirect_dma_start(
                            out=g[:, j, :], out_offset=None, in_=h_l[:],
                            in_offset=bass.IndirectOffsetOnAxis(
                                ap=fidx_loc_t[:, col:col + 1], axis=0),
                            bounds_check=bnd_sh, oob_is_err=False,
                        )
                stage = stp.tile([P, PC, 576], F16, tag="stage")
                for k in range(9):
                    srcb = g[:, :, k * D:(k + 1) * D]
                    dst = _dims(stage[:, :, k:k + 1],
                                [stage[:].ap[0], [576, PC], [9, D]])
                    if k % 2 == 0:
                        nc.vector.tensor_copy(out=dst, in_=srcb)
                    else:
                        nc.scalar.activation(dst, srcb, COPY)
                obase = out_f[pc_i * PC * P:(pc_i + 1) * PC * P, :]
                orows = _dims(obase, [[576, P], [P * 576, PC], [1, 576]])
                nc.sync.dma_start(out=orows, in_=stage[:])

    _split_multi_waits(nc)
    return nc


def _plan(x, deg, adj_row, adj_col, edge):
    """Host-side sharding: pure index bookkeeping + input reordering."""
    x = np.asarray(x, np.float32)
    deg = np.asarray(deg, np.float32).reshape(-1)
    adj_row = np.asarray(adj_row, np.int64)
    adj_col = np.asarray(adj_col, np.int64)
    edge = np.asarray(edge, np.int64)

    iota_np = np.tile(np.arange(P, dtype=np.float16), (P, 1))
    ep = edge.reshape(-1)

    in_maps, positions = [], []
    for c in range(NC):
        r0 = c * NSH
        idx1 = np.full((P, EDGE_COLS), BIG, np.int32)
        rowloc = np.full((P, EDGE_COLS), -1.0, np.float16)
        for t in range(NT):
            base = r0 + t * P
            lo = np.searchsorted(adj_row, base, side="left")
            hi = np.searchsorted(adj_row, base + P, side="left")
            n_e = hi - lo
            assert n_e <= NCHUNK * P, f"tile overflow: {n_e}"
            sl = np.arange(n_e)
            jj, pp_ = divmod(sl, P)
            colbase = t * NCHUNK
            idx1[pp_, colbase + jj] = adj_col[lo:hi]
            rowloc[pp_, colbase + jj] = (adj_row[lo:hi] - base).astype(np.float16)
        real = min(NSH, max(0, N - r0))
        dlocal = np.ones(NSH, np.float32)
        dlocal[:real] = deg[r0:r0 + real]
        degsh = dlocal.reshape(NT, P).T.copy()

        x_shard = np.zeros((NSH, D), np.float32)
        x_shard[:real] = x[r0:r0 + real]

        mine = np.nonzero((ep >= r0) & (ep < r0 + NSH))[0]
        n_c = len(mine)
        assert n_c <= FCH * P, f"endpoint overflow: {n_c}"
        fidx_loc = np.full((P, FCH), BIG, np.int32)
        sl = np.arange(n_c)
        jj, pp_ = divmod(sl, P)
        fidx_loc[pp_, jj] = (ep[mine] - r0).astype(np.int32)
        positions.append(mine)

        in_maps.append({
            "x_sh": x_shard,
            "degsh": degsh,
            "idx1": idx1,
            "rowloc": rowloc,
            "fidx_loc": fidx_loc,
            "iota": iota_np,
        })
    return in_maps, positions


def _assemble(results, positions):
    out = np.zeros((2 * EQ, 576), np.float32)
    for c in range(NC):
        rows = results[c]["out_f"]
        n_c = len(positions[c])
        out[positions[c]] = rows[:n_c].astype(np.float32)
    return out.reshape(2, EQ, D, 9)


def kernel(x, deg, adj_row, adj_col, edge):
    import time
    if "nc" not in _prog_cache:
        t0 = time.time()
        _prog_cache["nc"] = _build_program()
        print(f"[kernel] program build: {time.time()-t0:.1f}s", flush=True)
    nc = _prog_cache["nc"]
    t0 = time.time()
    in_maps, positions = _plan(x, deg, adj_row, adj_col, edge)
    print(f"[kernel] host plan: {time.time()-t0:.1f}s", flush=True)
    t0 = time.time()
    res = run_bass_kernel_spmd(nc, in_maps, list(range(NC)))
    print(f"[kernel] compile+run: {time.time()-t0:.1f}s", flush=True)
    return _assemble(res.results, positions)


# revision 23
# speedup vs baseline: 1.0884x; 1.0486x over previous
"""Distributed Trainium2 Bass kernel for nn_NodeFeat (2-hop Chebyshev-style GNN
feature expansion + edge gather), 8 NeuronCores.

Node sharding per the problem's sharding hint:
  - 50000 nodes padded to 50176 = 8 x 6272; core c owns rows [6272c, 6272c+6272).
  - adjacency rows are pre-sorted; each core handles the edges whose ROW is in
    its shard, packed per 128-row dst tile into NCHUNK=18 chunks of 128 slots
    (dummy slots gather row 0; their one-hot selector columns are zero).
  - pre-phase: each core computes its x3 = [x | x*rsqrt(deg) | x*sqrt(deg)]
    shard in fp16 (per-NODE degree scaling, so no per-edge degree work later);
    one AllGather publishes the fp16 x3_full table.
  - hop1: per dst tile, indirect-DMA gathers fetch the up-to-18x128 neighbor
    rows of x3_full (384B fp16 rows); segment-sum on TensorE via a one-hot
    selector (is_equal of rowloc vs iota) accumulated in PSUM; ScalarE
    evacuates with the 1/deg row scale straight to fp16.
  - one AllGather of the per-core y1 shard (fp16) between hops.
  - hop2: same machinery gathering y1full rows, then minus x3 (kept in SBUF).
  - final: edge endpoints partitioned by owner core (host all-to-all
    bookkeeping); all three tables (x3, y1, xs2) are core-local, so each core
    gathers its endpoint rows locally, transposes [9,64] -> [64,9] on-chip,
    writes packed fp16 rows; the host scatters into the [2,32768,64,9] f32
    output (fp16 -> f32 upcast is exact).

All floating-point math runs on device; the host only shards, pads, reorders
and reassembles (index bookkeeping).
"""
import numpy as np

import concourse.bass as bass
import concourse.mybir as mybir
import concourse.tile as tile
from concourse.bass_utils import run_bass_kernel_spmd

# ---------------- hardcoded problem geometry ----------------
N = 50000
D = 64
EQ = 32768
P = 128
NC = 8                   # cores
NT = 49                  # row tiles per core
NSH = NT * P             # 6272 rows per core
NPAD = NSH * NC          # 50176
NCHUNK = 18              # 128-edge chunks per row tile
FCH = 66                 # final-gather chunks per core (66*128 = 8448 slots)
PC = 6                   # final-gather chunks per piece (11 pieces)
CPAD = 192               # fp16 cols per table row
F32 = mybir.dt.float32
F16 = mybir.dt.float16
I32 = mybir.dt.int32
EDGE_COLS = NT * NCHUNK  # 882

_prog_cache = {}


class _TC(tile.TileContext):
    """TileContext whose final drain splits sem waits one-per-instruction
    (this walrus rejects >1 sync wait on an instruction)."""

    def _drain_and_barrier(self, tick_clock, wait_clock):
        nc = self.nc
        probe = nc.sync.nop()
        wait_clock.add_sem_waits(
            probe.ins, tile.ScopedClock({None: tick_clock.global_clock}))
        si = probe.ins.sync_info
        waits = list(si.on_wait) if si and si.on_wait else []
        if si is not None:
            si.on_wait = waits[:1]
        for w in waits[1:]:
            n2 = nc.sync.nop()
            if n2.ins.sync_info is None:
                n2.ins.sync_info = mybir.SyncInfo(on_wait=[w], on_update=[])
            else:
                n2.ins.sync_info.on_wait = [w]
        nc.sync.drain()
        nc.all_engine_barrier()
        popped = nc._tile_sem_poison_stack.pop()
        assert popped is self._sem_poison
        nc.clear_and_free_semaphores(list(self.sems.allocated().values()))
        nc.all_engine_barrier()


def _split_multi_waits(nc):
    for fn in nc.m.functions:
        for blk in fn.blocks:
            new_list = []
            for inst in blk.instructions:
                si = inst.sync_info
                waits = list(si.on_wait) if si and si.on_wait else []
                if len(waits) > 1:
                    for j, w in enumerate(waits[:-1]):
                        nop = mybir.InstNoOp(
                            name=f"{inst.name}-ws{j}",
                            engine=inst.engine,
                            ins=[], outs=[],
                            sync_info=mybir.SyncInfo(on_wait=[w], on_update=[]),
                        )
                        nc.register_instruction(nop, overwrite=True)
                        new_list.append(nop)
                    si.on_wait = waits[-1:]
                new_list.append(inst)
            blk.instructions[:] = new_list


def _dims(ap, dims):
    """Same tensor+offset as `ap`, explicit [stride(elem), nelem] dims."""
    return bass.AP(ap.tensor, ap.offset, dims)


def _build_program(ablate=()):
    """ablate: subset of {"pre","hop1","gather1","ag","hop2","gather2",
    "final","gatherf"} to SKIP (perf ablation only — results become wrong)."""
    ab = set(ablate)
    nc = bass.Bass("TRN2", target_bir_lowering=False, debug=False, num_devices=NC)

    x_sh = nc.dram_tensor("x_sh", [NSH, D], F32, kind="ExternalInput")
    degsh_in = nc.dram_tensor("degsh", [P, NT], F32, kind="ExternalInput")
    idx1_in = nc.dram_tensor("idx1", [P, EDGE_COLS], I32, kind="ExternalInput")
    rowloc_in = nc.dram_tensor("rowloc", [P, EDGE_COLS], F16, kind="ExternalInput")
    fidx_loc_in = nc.dram_tensor("fidx_loc", [P, FCH], I32, kind="ExternalInput")
    iota_in = nc.dram_tensor("iota", [P, P], F16, kind="ExternalInput")

    out_f = nc.dram_tensor("out_f", [FCH * P, 576], F16, kind="ExternalOutput")

    x3_bounce = nc.dram_tensor("x3_bounce", [NSH, CPAD], F16)
    x3_full = nc.dram_tensor("x3_full", [NPAD, CPAD], F16, addr_space="Shared")
    y1_bounce = nc.dram_tensor("y1_bounce", [NSH, CPAD], F16)
    y1full = nc.dram_tensor("y1full", [NPAD, CPAD], F16, addr_space="Shared")
    h_l = nc.dram_tensor("h_l", [NSH, 576], F16)

    eq = mybir.AluOpType.is_equal
    mult = mybir.AluOpType.mult
    sub = mybir.AluOpType.subtract
    COPY = mybir.ActivationFunctionType.Copy
    SQRT = mybir.ActivationFunctionType.Sqrt

    with _TC(nc) as tc, nc.allow_low_precision(reason="fp16 tables and matmul operands; PSUM accumulates in f32"):
        with (
            tc.tile_pool(name="const", bufs=1) as cp,
            tc.tile_pool(name="s", bufs=4) as sp_,
            tc.tile_pool(name="v3", bufs=4) as v3p,
            tc.tile_pool(name="ev", bufs=3) as evp,
            tc.tile_pool(name="x0", bufs=3) as x0p,
            tc.tile_pool(name="v2", bufs=4) as v2p,
            tc.tile_pool(name="g", bufs=3) as gp,
            tc.tile_pool(name="st", bufs=3) as stp,
            tc.tile_pool(name="psum", bufs=4, space="PSUM") as pp,
        ):
            iota_t = cp.tile([P, P], F16)
            nc.sync.dma_start(out=iota_t[:], in_=iota_in[:])
            idx1_t = cp.tile([P, EDGE_COLS], I32)
            nc.sync.dma_start(out=idx1_t[:], in_=idx1_in[:])
            rowloc_t = cp.tile([P, EDGE_COLS], F16)
            nc.sync.dma_start(out=rowloc_t[:], in_=rowloc_in[:])
            degsh_t = cp.tile([P, NT], F32)
            nc.sync.dma_start(out=degsh_t[:], in_=degsh_in[:])
            fidx_loc_t = cp.tile([P, FCH], I32)
            nc.sync.dma_start(out=fidx_loc_t[:], in_=fidx_loc_in[:])

            def build_s(t):
                s_t = sp_.tile([P, NCHUNK, P], F16, tag="s")
                rl = rowloc_t[:, t * NCHUNK:(t + 1) * NCHUNK]
                rl_b = rl.to_broadcast([P, NCHUNK, P])
                io = iota_t[:]
                io_b = _dims(io, [io.ap[0], [0, NCHUNK], io.ap[1]])
                nc.vector.tensor_tensor(out=s_t[:], in0=rl_b, in1=io_b, op=eq)
                return s_t

            def hop_gather(v, table, t):
                for j in range(NCHUNK):
                    col = t * NCHUNK + j
                    nc.gpsimd.indirect_dma_start(
                        out=v[:, j, :], out_offset=None, in_=table[:],
                        in_offset=bass.IndirectOffsetOnAxis(
                            ap=idx1_t[:, col:col + 1], axis=0),
                    )

            # whole-shard precomputes
            degrev_all = cp.tile([P, NT], F32)
            nc.vector.reciprocal(degrev_all[:], degsh_t[:])
            rq0_all = cp.tile([P, 2, NT], F32)
            nc.scalar.activation(rq0_all[:, 1, :], degsh_t[:], SQRT)
            nc.vector.reciprocal(rq0_all[:, 0, :], rq0_all[:, 1, :])
            # SBUF-resident x3 of the own shard (fp16): [P, NT, 192]
            x3_own = cp.tile([P, NT, 192], F16)

            # ============ pre-phase: x3 = [x | x*rsqrt(deg) | x*sqrt(deg)] ====
            for t in range(NT if "pre" not in ab else 0):
                x_t = x0p.tile([P, D], F32, tag="xt")
                nc.sync.dma_start(out=x_t[:], in_=x_sh[t * P:(t + 1) * P, :])
                nc.scalar.activation(x3_own[:, t, 0:D], x_t[:], COPY)
                b12 = _dims(x3_own[:, t, D:3 * D],
                            [x3_own[:].ap[0], [D, 2], [1, D]])
                xb = _dims(x_t[:], [x_t[:].ap[0], [0, 2], [1, D]])
                rq0b = _dims(rq0_all[:, :, t:t + 1],
                             [rq0_all[:].ap[0], [NT, 2], [0, D]])
                nc.vector.tensor_tensor(out=b12, in0=xb, in1=rq0b, op=mult)
                nc.sync.dma_start(out=x3_bounce[t * P:(t + 1) * P, :],
                                  in_=x3_own[:, t, :])
                h0 = _dims(h_l[t * P:(t + 1) * P, 0:CPAD],
                           [[576, P], [1, CPAD]])
                nc.sync.dma_start(out=h0, in_=x3_own[:, t, :])

            # ================= AllGather x3 =================
            if "ag" not in ab:
                nc.gpsimd.collective_compute(
                    "AllGather", mybir.AluOpType.bypass,
                    replica_groups=[list(range(NC))],
                    ins=[x3_bounce[:]], outs=[x3_full[:]],
                )

            # ================= hop 1 =================
            for t in range(NT if "hop1" not in ab else 0):
                v3 = v3p.tile([P, NCHUNK, CPAD], F16, tag="v3")
                if t < 4 or "gather1" in ab:
                    nc.vector.memset(v3[:], 0.0)
                if "gather1" not in ab:
                    hop_gather(v3, x3_full, t)
                s_t = build_s(t)
                ps = pp.tile([P, 192], F32, space="PSUM", tag="ps")
                for j in range(NCHUNK):
                    nc.tensor.matmul(
                        out=ps[:], lhsT=s_t[:, j, :], rhs=v3[:, j, 0:192],
                        start=(j == 0), stop=(j == NCHUNK - 1))
                y1_t = evp.tile([P, 192], F16, tag="y1")
                nc.scalar.activation(y1_t[:], ps[:], COPY,
                                     scale=degrev_all[:, t:t + 1])
                nc.sync.dma_start(out=y1_bounce[t * P:(t + 1) * P, :], in_=y1_t[:])
                h1 = _dims(h_l[t * P:(t + 1) * P, CPAD:2 * CPAD],
                           [[576, P], [1, CPAD]])
                nc.sync.dma_start(out=h1, in_=y1_t[:])

            # ================= AllGather y1 =================
            if "ag" not in ab:
                nc.gpsimd.collective_compute(
                    "AllGather", mybir.AluOpType.bypass,
                    replica_groups=[list(range(NC))],
                    ins=[y1_bounce[:]], outs=[y1full[:]],
                )

            # ================= hop 2 =================
            for t in range(NT if "hop2" not in ab else 0):
                v2 = v2p.tile([P, NCHUNK, CPAD], F16, tag="v2")
                if t < 4 or "gather2" in ab:
                    nc.vector.memset(v2[:], 0.0)
                if "gather2" not in ab:
                    hop_gather(v2, y1full, t)
                s_t = build_s(t)
                ps = pp.tile([P, 192], F32, space="PSUM", tag="ps")
                for j in range(NCHUNK):
                    nc.tensor.matmul(
                        out=ps[:], lhsT=s_t[:, j, :], rhs=v2[:, j, 0:192],
                        start=(j == 0), stop=(j == NCHUNK - 1))
                tmp = evp.tile([P, 192], F16, tag="tmp2")
                nc.scalar.activation(tmp[:], ps[:], COPY,
                                     scale=degrev_all[:, t:t + 1])
                xs2_t = evp.tile([P, 192], F16, tag="xs2")
                nc.vector.tensor_tensor(out=xs2_t[:], in0=tmp[:],
                                        in1=x3_own[:, t, :], op=sub)
                h2 = _dims(h_l[t * P:(t + 1) * P, 2 * CPAD:3 * CPAD],
                           [[576, P], [1, CPAD]])
                nc.sync.dma_start(out=h2, in_=xs2_t[:])

            # ================= final gather + transpose =================
            for pc_i in range(FCH // PC if "final" not in ab else 0):
                g = gp.tile([P, PC, 576], F16, tag="g")
                if pc_i < 3 or "gatherf" in ab:
                    nc.vector.memset(g[:], 0.0)
                if "gatherf" not in ab:
                    for j in range(PC):
                        col = pc_i * PC + j
                        nc.gpsimd.indirect_dma_start(
                            out=g[:, j, :], out_offset=None, in_=h_l[:],
                            in_offset=bass.IndirectOffsetOnAxis(
                                ap=fidx_loc_t[:, col:col + 1], axis=0),
                        )
                stage = stp.tile([P, PC, 576], F16, tag="stage")
                for k in range(9):
                    srcb = g[:, :, k * D:(k + 1) * D]
                    dst = _dims(stage[:, :, k:k + 1],
                                [stage[:].ap[0], [576, PC], [9, D]])
                    if k % 2 == 0:
                        nc.vector.tensor_copy(out=dst, in_=srcb)
                    else:
                        nc.scalar.activation(dst, srcb, COPY)
                obase = out_f[pc_i * PC * P:(pc_i + 1) * PC * P, :]
                orows = _dims(obase, [[576, P], [P * 576, PC], [1, 576]])
                nc.sync.dma_start(out=orows, in_=stage[:])

    _split_multi_waits(nc)
    return nc


def _plan(x, deg, adj_row, adj_col, edge):
    """Host-side sharding: pure index bookkeeping + input reordering."""
    x = np.asarray(x, np.float32)
    deg = np.asarray(deg, np.float32).reshape(-1)
    adj_row = np.asarray(adj_row, np.int64)
    adj_col = np.asarray(adj_col, np.int64)
    edge = np.asarray(edge, np.int64)

    iota_np = np.tile(np.arange(P, dtype=np.float16), (P, 1))
    ep = edge.reshape(-1)

    in_maps, positions = [], []
    for c in range(NC):
        r0 = c * NSH
        idx1 = np.zeros((P, EDGE_COLS), np.int32)
        rowloc = np.full((P, EDGE_COLS), -1.0, np.float16)
        for t in range(NT):
            base = r0 + t * P
            lo = np.searchsorted(adj_row, base, side="left")
            hi = np.searchsorted(adj_row, base + P, side="left")
            n_e = hi - lo
            assert n_e <= NCHUNK * P, f"tile overflow: {n_e}"
            sl = np.arange(n_e)
            jj, pp_ = divmod(sl, P)
            colbase = t * NCHUNK
            idx1[pp_, colbase + jj] = adj_col[lo:hi]
            rowloc[pp_, colbase + jj] = (adj_row[lo:hi] - base).astype(np.float16)
        real = min(NSH, max(0, N - r0))
        dlocal = np.ones(NSH, np.float32)
        dlocal[:real] = deg[r0:r0 + real]
        degsh = dlocal.reshape(NT, P).T.copy()

        x_shard = np.zeros((NSH, D), np.float32)
        x_shard[:real] = x[r0:r0 + real]

        mine = np.nonzero((ep >= r0) & (ep < r0 + NSH))[0]
        n_c = len(mine)
        assert n_c <= FCH * P, f"endpoint overflow: {n_c}"
        fidx_loc = np.zeros((P, FCH), np.int32)
        sl = np.arange(n_c)
        jj, pp_ = divmod(sl, P)
        fidx_loc[pp_, jj] = (ep[mine] - r0).astype(np.int32)
        positions.append(mine)

        in_maps.append({
            "x_sh": x_shard,
            "degsh": degsh,
            "idx1": idx1,
            "rowloc": rowloc,
            "fidx_loc": fidx_loc,
            "iota": iota_np,
        })
    return in_maps, positions


def _assemble(results, positions):
    out = np.zeros((2 * EQ, 576), np.float32)
    for c in range(NC):
        rows = results[c]["out_f"]
        n_c = len(positions[c])
        out[positions[c]] = rows[:n_c].astype(np.float32)
    return out.reshape(2, EQ, D, 9)


def kernel(x, deg, adj_row, adj_col, edge):
    import time
    if "nc" not in _prog_cache:
        t0 = time.time()
        _prog_cache["nc"] = _build_program()
        print(f"[kernel] program build: {time.time()-t0:.1f}s", flush=True)
    nc = _prog_cache["nc"]
    t0 = time.time()
    in_maps, positions = _plan(x, deg, adj_row, adj_col, edge)
    print(f"[kernel] host plan: {time.time()-t0:.1f}s", flush=True)
    t0 = time.time()
    res = run_bass_kernel_spmd(nc, in_maps, list(range(NC)))
    print(f"[kernel] compile+run: {time.time()-t0:.1f}s", flush=True)
    return _assemble(res.results, positions)
